# revision 10
# baseline (speedup 1.0000x reference)
"""Trainium2 Bass kernel for LGeM self-attention (b=2, t=2048, c=2048, h=16, d=128).

Sharding: 8 cores = 2 (batch, data-parallel) x 4 (head-groups of 4 heads,
tensor-parallel 'mp'). Each core computes q/k/v projections for its 4 heads,
attention, and a partial output projection (its 512 rows of Wo); the host
sums the 4 mp-partials per batch in fp32.

v2 design (vs the fp32 DRAM-scratch baseline):
  - fp16 storage for every matmul operand (x, packed qkv weights, wo, q, k,
    v, exp(S), out2, out partials); fp32 PSUM accumulation throughout. Halves
    HBM traffic and SBUF footprint, enables fast weight load on the PE.
  - Everything stays in SBUF between phases - no DRAM scratch round trips.
  - Host packs x / wqkv / wo into the exact SBUF layouts, so the input side
    is ~34 large contiguous DMAs arriving at contraction-chunk granularity
    (accumulation chains advance as the stream lands); output is 16 row-block
    stores issued on the gpsimd (SWDGE) queue so they don't contend with the
    input HWDGE ring.
  - softmax denominator: running DVE fp16 accumulation of the exp tiles
    (esum += et), then a single ones[128,128] matmul per (head, tq-chunk)
    replicates the partition-sum across partitions (free broadcast for the
    reciprocal multiply). Saves 240 PE matmuls vs accumulating ones@et per
    key tile.
  - scores are built transposed, S_T[tk, tq] = kT_tile.T @ qT, so attn@v
    needs no transposes and the normalized context lands as out2T[d, tq] -
    exactly the lhsT the output projection wants.
  - rope here is q*(cos+sin) elementwise (the module's rotate_half is
    identity); the 1/sqrt(t) logit scale is folded into the shared cf tensor
    as T**-0.25 (applied to both q and k). softmax is computed without
    max-subtraction (logits are ~N(0, 0.25^2): exp never overflows).
  - loop order tq-outer / head-inner in attention, with the output
    projection for each tq chunk emitted right after its last head, so
    out-proj matmuls fill PE gaps while ACT works on the next chunk's exps.
"""

import sys

sys.path.insert(0, "/opt/trn_rl_repo")

import numpy as np

import concourse.bass as bass
import concourse.mybir as mybir
import concourse.tile as tile
from concourse import bacc, bass_utils

F32 = mybir.dt.float32
F16 = mybir.dt.float16

HIDDEN = 2048
HEADS = 16
HEAD_DIM = 128
SEQ = 2048
BATCH = 2
N_CORES = 8
MP = 4  # tensor-parallel cores per batch
HG = HEADS // MP  # heads per core
THETA = 10000.0

CCH = HIDDEN // 128  # 16 contraction chunks for projections
XG = 4  # x / w DMA groups
CCG = CCH // XG  # cc chunks per group
TQC = 512  # moving-dim chunk (tq)
NTQ = SEQ // TQC
NTK = SEQ // 128
DG = HG * HEAD_DIM  # 512 projection cols per core


def build_attention_nc(use_mask=False):
    T, C, D = SEQ, HIDDEN, HEAD_DIM
    EXP = mybir.ActivationFunctionType.Exp

    nc = bacc.Bacc("TRN2", target_bir_lowering=False, debug=False)

    # [p, cc*T + t] = x[t, cc*128+p]
    xp = nc.dram_tensor("xp", [128, CCH * T], F16, kind="ExternalInput").ap()
    # [p, cc*1536 + wi*512 + dcol] = W_wi[cc*128+p, dcol]  (wi: 0=q,1=k,2=v)
    wqkv = nc.dram_tensor("wqkv", [128, CCH * 3 * DG], F16, kind="ExternalInput").ap()
    # [p, h*C + c] = Wo[h*128+p, c]
    wop = nc.dram_tensor("wop", [128, HG * C], F16, kind="ExternalInput").ap()
    cfd = nc.dram_tensor("cf", [D, T], F32, kind="ExternalInput").ap()
    if use_mask:
        maskT = nc.dram_tensor("maskT", [T, T], F32, kind="ExternalInput").ap()
    out = nc.dram_tensor("out", [T, C], F16, kind="ExternalOutput").ap()

    with tile.TileContext(nc) as tc:
        with tc.tile_pool(name="persist", bufs=1) as pp:
            qT = pp.tile([128, HG * T], F16, tag="qT")  # [d, h*T + t]
            kT = pp.tile([128, HG * T], F16, tag="kT")
            vv = pp.tile([128, NTK * DG], F16, tag="vv")  # [t%128, tk*DG + dg]
            o2 = pp.tile([128, HG * T], F16, tag="o2")  # [d, h*T + t]
            cfs = pp.tile([128, T], F32, tag="cfs")
            ones = pp.tile([128, 128], F16, tag="ones")
            nc.vector.memset(ones[:], 1.0)

            # ---------------- Phase A: projections ----------------
            with tc.tile_pool(name="xw", bufs=1) as xw:
                xg = [
                    xw.tile([128, CCG * T], F16, tag=f"xg{g}", name=f"xg{g}")
                    for g in range(XG)
                ]
                wg = [
                    xw.tile([128, CCG * 3 * DG], F16, tag=f"wg{g}", name=f"wg{g}")
                    for g in range(XG)
                ]
                # cc-granular arrival so accumulation chains advance smoothly
                # while the input stream lands (avoids group-sized PE stalls)
                for cc in range(CCH):
                    g, j = divmod(cc, CCG)
                    nc.sync.dma_start(
                        xg[g][:, j * T : (j + 1) * T],
                        xp[:, cc * T : (cc + 1) * T],
                    )
                    nc.sync.dma_start(
                        wg[g][:, j * 3 * DG : (j + 1) * 3 * DG],
                        wqkv[:, cc * 3 * DG : (cc + 1) * 3 * DG],
                    )
                    if cc == 1:
                        # cf is first needed by the rope multiply after the
                        # first 16-matmul chain; don't let it delay x0/w0
                        nc.sync.dma_start(cfs[:], cfd)

                with (
                    tc.tile_pool(name="vps", bufs=4, space="PSUM") as vps,
                    tc.tile_pool(name="qkps", bufs=2, space="PSUM") as qkps,
                ):

                    def emit_qk(h):
                        # two tq chunks share one 2-bank PSUM tile: the
                        # stationary weight chunk is reused across both
                        # matmuls and the rope multiply runs 1024 wide.
                        for wi, dst in ((0, qT), (1, kT)):
                            for tqp in range(NTQ // 2):
                                pm = qkps.tile([128, 2 * TQC], F32, tag="pm")
                                for cc in range(CCH):
                                    g, j = divmod(cc, CCG)
                                    woff = j * 3 * DG + wi * DG + h * D
                                    for half in range(2):
                                        tq = 2 * tqp + half
                                        nc.tensor.matmul(
                                            pm[:, half * TQC : (half + 1) * TQC],
                                            wg[g][:, woff : woff + D],
                                            xg[g][
                                                :,
                                                j * T + tq * TQC : j * T
                                                + (tq + 1) * TQC,
                                            ],
                                            start=(cc == 0),
                                            stop=(cc == CCH - 1),
                                        )
                                nc.vector.tensor_mul(
                                    dst[
                                        :,
                                        h * T + 2 * tqp * TQC : h * T
                                        + 2 * (tqp + 1) * TQC,
                                    ],
                                    pm[:],
                                    cfs[:, 2 * tqp * TQC : 2 * (tqp + 1) * TQC],
                                )

                    def emit_v():
                        for tk in range(NTK):
                            pv = vps.tile([128, DG], F32, tag="pv")
                            for cc in range(CCH):
                                g, j = divmod(cc, CCG)
                                nc.tensor.matmul(
                                    pv[:],
                                    xg[g][:, j * T + tk * 128 : j * T + tk * 128 + 128],
                                    wg[g][:, j * 3 * DG + 2 * DG : (j + 1) * 3 * DG],
                                    start=(cc == 0),
                                    stop=(cc == CCH - 1),
                                )
                            nc.any.tensor_copy(vv[:, tk * DG : (tk + 1) * DG], pv[:])

                    emit_qk(0)
                    emit_v()
                    for h in range(1, HG):
                        emit_qk(h)

            # ---------------- Phase B: attention + output projection ----------
            with (
                tc.tile_pool(name="phb", bufs=1) as bp,
                tc.tile_pool(name="ep", bufs=6) as epool,
                tc.tile_pool(name="es", bufs=3) as espool,
                tc.tile_pool(name="rp", bufs=2) as rpool,
                tc.tile_pool(name="op", bufs=3) as opool,
                tc.tile_pool(name="mp", bufs=4) as mpool,
                tc.tile_pool(name="scps", bufs=2, space="PSUM") as scps,
                tc.tile_pool(name="o2ps", bufs=2, space="PSUM") as o2ps,
                tc.tile_pool(name="accps", bufs=2, space="PSUM") as accps,
            ):
                wos = bp.tile([128, HG * C], F16, tag="wos")
                nc.sync.dma_start(wos[:], wop)

                def attn_chunk(h, tq):
                    """Scores/exp/AV for one (head, tq) chunk. Two key tiles
                    share one 2-bank scores PSUM tile so the exp runs 1024
                    wide (amortizes ACT per-instruction overhead). Returns the
                    state needed by finalize(), which is emitted later (one
                    head behind) so the softmax tail chain (esum -> ones-mm ->
                    recip -> mul) never blocks the PE stream."""
                    o2p = o2ps.tile([128, TQC], F32, tag="o2p")
                    esum = espool.tile([128, TQC], F16, tag="esum")
                    qrhs = qT[:, h * T + tq * TQC : h * T + (tq + 1) * TQC]
                    pend = None
                    for tkp in range(NTK // 2):
                        scp = scps.tile([128, 2 * TQC], F32, tag="scp")
                        for half in range(2):
                            tk = 2 * tkp + half
                            nc.tensor.matmul(
                                scp[:, half * TQC : (half + 1) * TQC],
                                kT[:, h * T + tk * 128 : h * T + tk * 128 + 128],
                                qrhs,
                                start=True,
                                stop=True,
                            )
                        et = epool.tile([128, 2 * TQC], F16, tag="et")
                        if use_mask:
                            ma = mpool.tile([128, 2 * TQC], F32, tag="ma")
                            for half in range(2):
                                tk = 2 * tkp + half
                                mt = mpool.tile([128, TQC], F32, tag="mt")
                                nc.sync.dma_start(
                                    mt[:],
                                    maskT[
                                        tk * 128 : (tk + 1) * 128,
                                        tq * TQC : (tq + 1) * TQC,
                                    ],
                                )
                                nc.vector.tensor_add(
                                    ma[:, half * TQC : (half + 1) * TQC],
                                    scp[:, half * TQC : (half + 1) * TQC],
                                    mt[:],
                                )
                            nc.scalar.activation(et[:], ma[:], EXP)
                        else:
                            nc.scalar.activation(et[:], scp[:], EXP)
                        for half in range(2):
                            if tkp == 0 and half == 0:
                                nc.vector.tensor_copy(
                                    esum[:], et[:, half * TQC : (half + 1) * TQC]
                                )
                            else:
                                nc.vector.tensor_add(
                                    esum[:],
                                    esum[:],
                                    et[:, half * TQC : (half + 1) * TQC],
                                )
                        if pend is not None:
                            p_et, ptkp = pend
                            for half in range(2):
                                ptk = 2 * ptkp + half
                                nc.tensor.matmul(
                                    o2p[:],
                                    vv[:, ptk * DG + h * D : ptk * DG + (h + 1) * D],
                                    p_et[:, half * TQC : (half + 1) * TQC],
                                    start=(ptk == 0),
                                    stop=False,
                                )
                        pend = (et, tkp)
                    p_et, ptkp = pend
                    for half in range(2):
                        ptk = 2 * ptkp + half
                        nc.tensor.matmul(
                            o2p[:],
                            vv[:, ptk * DG + h * D : ptk * DG + (h + 1) * D],
                            p_et[:, half * TQC : (half + 1) * TQC],
                            start=False,
                            stop=(half == 1),
                        )
                    return (h, tq, o2p, esum)

                def finalize(state):
                    h, tq, o2p, esum = state
                    sp = accps.tile([128, TQC], F32, tag="acc")
                    nc.tensor.matmul(sp[:], ones[:], esum[:], start=True, stop=True)
                    rt = rpool.tile([128, TQC], F32, tag="rt")
                    nc.vector.reciprocal(rt[:], sp[:])
                    nc.vector.tensor_mul(
                        o2[:, h * T + tq * TQC : h * T + (tq + 1) * TQC],
                        o2p[:],
                        rt[:],
                    )

                def outproj_qt(qt):
                    ost = opool.tile([128, C], F16, tag="ost")
                    for oc in range(C // TQC):
                        fp = accps.tile([128, TQC], F32, tag="acc")
                        for h in range(HG):
                            nc.tensor.matmul(
                                fp[:],
                                o2[:, h * T + qt * 128 : h * T + qt * 128 + 128],
                                wos[:, h * C + oc * TQC : h * C + (oc + 1) * TQC],
                                start=(h == 0),
                                stop=(h == HG - 1),
                            )
                        nc.vector.tensor_copy(ost[:, oc * TQC : (oc + 1) * TQC], fp[:])
                    nc.gpsimd.dma_start(out[qt * 128 : (qt + 1) * 128, :], ost[:])

                # out-proj of tq chunk N is spread one row-tile per attention
                # chunk of tq N+1, so its matmuls fill PE slack while ACT
                # works on fresh exps; the last chunk's row-tiles drain at the
                # end.
                pending_fin = None
                for tq in range(NTQ):
                    for h in range(HG):
                        st = attn_chunk(h, tq)
                        if pending_fin is not None:
                            finalize(pending_fin)
                        pending_fin = st
                        if tq > 0:
                            outproj_qt((tq - 1) * 4 + h)
                    finalize(pending_fin)
                    pending_fin = None
                for qt in range((NTQ - 1) * 4, NTQ * 4):
                    outproj_qt(qt)

    nc.compile()
    return nc


def compute_cf(T, D, theta=THETA):
    """cf = (cos+sin).T * T**-0.25  [D, T] fp32.

    Both q and k are multiplied by cf, so scaling cf by T**-0.25 applies the
    module's 1/sqrt(T) logit scale symmetrically. Folding it here (instead of
    into Wq) keeps the fp16 weight values out of subnormal range."""
    freq = 1.0 / theta ** (np.arange(0, D, 2, dtype=np.float64) / D)
    t = np.arange(T, dtype=np.float64)
    m = np.einsum("i,j->ij", t, freq)  # [T, D/2]
    m = np.concatenate([m, m], axis=-1)  # [T, D]
    cfac = ((np.cos(m) + np.sin(m)) * T**-0.25).astype(np.float32)  # [T, D]
    return np.ascontiguousarray(cfac.T)  # [D, T]


_NC_CACHE = {}


def _get_nc(use_mask):
    key = bool(use_mask)
    if key not in _NC_CACHE:
        _NC_CACHE[key] = build_attention_nc(use_mask=key)
    return _NC_CACHE[key]


def _pack_x(x_b):
    """[T, C] fp32 -> [128, CCH*T] fp16 with [p, cc*T + t] = x[t, cc*128+p]."""
    xT = x_b.T.astype(np.float16)  # [C, T]
    return np.ascontiguousarray(
        xT.reshape(CCH, 128, SEQ).transpose(1, 0, 2).reshape(128, CCH * SEQ)
    )


def _pack_wqkv(wq_g, wk_g, wv_g):
    """three [C, DG] fp32 -> [128, CCH*3*DG] fp16."""
    parts = [w.reshape(CCH, 128, DG) for w in (wq_g, wk_g, wv_g)]
    w = np.concatenate(parts, axis=2)  # [CCH, 128, 3*DG]
    return np.ascontiguousarray(
        w.transpose(1, 0, 2).reshape(128, CCH * 3 * DG).astype(np.float16)
    )


def _pack_wo(wo_g):
    """[DG, C] fp32 -> [128, HG*C] fp16 with [p, h*C + c] = wo[h*128+p, c]."""
    return np.ascontiguousarray(
        wo_g.reshape(HG, 128, HIDDEN).transpose(1, 0, 2).reshape(128, HG * HIDDEN)
    ).astype(np.float16)


def kernel(input_ids, attention_mask, Wq, Wk, Wv, Wo):
    input_ids = np.asarray(input_ids, dtype=np.float32)
    attention_mask = np.asarray(attention_mask, dtype=np.float32)
    Wq = np.asarray(Wq, dtype=np.float32)
    Wk = np.asarray(Wk, dtype=np.float32)
    Wv = np.asarray(Wv, dtype=np.float32)
    Wo = np.asarray(Wo, dtype=np.float32)

    b, t, c = input_ids.shape
    assert (b, t, c) == (BATCH, SEQ, HIDDEN)

    use_mask = bool(np.any(attention_mask))
    nc = _get_nc(use_mask)

    cf = compute_cf(SEQ, HEAD_DIM)

    xs = [_pack_x(input_ids[bi]) for bi in range(BATCH)]
    masks = (
        [np.ascontiguousarray(attention_mask[bi, 0].T) for bi in range(BATCH)]
        if use_mask
        else None
    )
    wqkvs = []
    wos = []
    for g in range(MP):
        sl = slice(g * DG, (g + 1) * DG)
        wqkvs.append(_pack_wqkv(Wq[:, sl], Wk[:, sl], Wv[:, sl]))
        wos.append(_pack_wo(Wo[sl, :]))

    in_maps = []
    for core in range(N_CORES):
        bi, g = divmod(core, MP)
        m = {
            "xp": xs[bi],
            "wqkv": wqkvs[g],
            "wop": wos[g],
            "cf": cf,
        }
        if use_mask:
            m["maskT"] = masks[bi]
        in_maps.append(m)

    res = bass_utils.run_bass_kernel_spmd(nc, in_maps, core_ids=list(range(N_CORES)))

    out = np.zeros((BATCH, SEQ, HIDDEN), dtype=np.float32)
    for bi in range(BATCH):
        acc = res.results[bi * MP]["out"].astype(np.float32)
        for g in range(1, MP):
            acc = acc + res.results[bi * MP + g]["out"].astype(np.float32)
        out[bi] = acc
    return out


# revision 11
# speedup vs baseline: 1.0039x; 1.0039x over previous
"""Trainium2 Bass kernel for LGeM self-attention (b=2, t=2048, c=2048, h=16, d=128).

Sharding: 8 cores = 2 (batch, data-parallel) x 4 (head-groups of 4 heads,
tensor-parallel 'mp'). Each core computes q/k/v projections for its 4 heads,
attention, and a partial output projection (its 512 rows of Wo); the host
sums the 4 mp-partials per batch in fp32.

v2 design (vs the fp32 DRAM-scratch baseline):
  - fp16 storage for every matmul operand (x, packed qkv weights, wo, q, k,
    v, exp(S), out2, out partials); fp32 PSUM accumulation throughout. Halves
    HBM traffic and SBUF footprint, enables fast weight load on the PE.
  - Everything stays in SBUF between phases - no DRAM scratch round trips.
  - Host packs x / wqkv / wo into the exact SBUF layouts, so the input side
    is ~34 large contiguous DMAs arriving at contraction-chunk granularity
    (accumulation chains advance as the stream lands); output is 16 row-block
    stores issued on the gpsimd (SWDGE) queue so they don't contend with the
    input HWDGE ring.
  - softmax denominator: running DVE fp16 accumulation of the exp tiles
    (esum += et), then a single ones[128,128] matmul per (head, tq-chunk)
    replicates the partition-sum across partitions (free broadcast for the
    reciprocal multiply). Saves 240 PE matmuls vs accumulating ones@et per
    key tile.
  - scores are built transposed, S_T[tk, tq] = kT_tile.T @ qT, so attn@v
    needs no transposes and the normalized context lands as out2T[d, tq] -
    exactly the lhsT the output projection wants.
  - rope here is q*(cos+sin) elementwise (the module's rotate_half is
    identity); the 1/sqrt(t) logit scale is folded into the shared cf tensor
    as T**-0.25 (applied to both q and k). softmax is computed without
    max-subtraction (logits are ~N(0, 0.25^2): exp never overflows).
  - loop order tq-outer / head-inner in attention, with the output
    projection for each tq chunk emitted right after its last head, so
    out-proj matmuls fill PE gaps while ACT works on the next chunk's exps.
"""

import sys

sys.path.insert(0, "/opt/trn_rl_repo")

import numpy as np

import concourse.bass as bass
import concourse.mybir as mybir
import concourse.tile as tile
from concourse import bacc, bass_utils

F32 = mybir.dt.float32
F16 = mybir.dt.float16

HIDDEN = 2048
HEADS = 16
HEAD_DIM = 128
SEQ = 2048
BATCH = 2
N_CORES = 8
MP = 4  # tensor-parallel cores per batch
HG = HEADS // MP  # heads per core
THETA = 10000.0

CCH = HIDDEN // 128  # 16 contraction chunks for projections
XG = 4  # x / w DMA groups
CCG = CCH // XG  # cc chunks per group
TQC = 512  # moving-dim chunk (tq)
NTQ = SEQ // TQC
NTK = SEQ // 128
DG = HG * HEAD_DIM  # 512 projection cols per core


def build_attention_nc(use_mask=False):
    T, C, D = SEQ, HIDDEN, HEAD_DIM
    EXP = mybir.ActivationFunctionType.Exp

    nc = bacc.Bacc("TRN2", target_bir_lowering=False, debug=False)

    # [p, cc*3584 + t] = x[t, cc*128+p] for t<2048, then the three weight
    # chunks W_wi[cc*128+p, dcol] (wi: 0=q,1=k,2=v) in the last 1536 cols
    CW = T + 3 * DG  # 3584 packed columns per cc chunk
    xw_d = nc.dram_tensor("xw", [128, CCH * CW], F16, kind="ExternalInput").ap()
    # [p, h*C + c] = Wo[h*128+p, c]
    wop = nc.dram_tensor("wop", [128, HG * C], F16, kind="ExternalInput").ap()
    cfd = nc.dram_tensor("cf", [D, T], F32, kind="ExternalInput").ap()
    if use_mask:
        maskT = nc.dram_tensor("maskT", [T, T], F32, kind="ExternalInput").ap()
    out = nc.dram_tensor("out", [T, C], F16, kind="ExternalOutput").ap()

    with tile.TileContext(nc) as tc:
        with tc.tile_pool(name="persist", bufs=1) as pp:
            qT = pp.tile([128, HG * T], F16, tag="qT")  # [d, h*T + t]
            kT = pp.tile([128, HG * T], F16, tag="kT")
            vv = pp.tile([128, NTK * DG], F16, tag="vv")  # [t%128, tk*DG + dg]
            o2 = pp.tile([128, HG * T], F16, tag="o2")  # [d, h*T + t]
            cfs = pp.tile([128, T], F32, tag="cfs")
            ones = pp.tile([128, 128], F16, tag="ones")
            nc.vector.memset(ones[:], 1.0)

            # ---------------- Phase A: projections ----------------
            with tc.tile_pool(name="xw", bufs=1) as xw:
                xwg = [
                    xw.tile([128, CCG * CW], F16, tag=f"xwg{g}", name=f"xwg{g}")
                    for g in range(XG)
                ]
                # cc-granular arrival so accumulation chains advance smoothly
                # while the input stream lands (avoids group-sized PE stalls)
                for cc in range(CCH):
                    g, j = divmod(cc, CCG)
                    nc.sync.dma_start(
                        xwg[g][:, j * CW : (j + 1) * CW],
                        xw_d[:, cc * CW : (cc + 1) * CW],
                    )
                    if cc == 1:
                        # cf is first needed by the rope multiply after the
                        # first 16-matmul chain; don't let it delay x0/w0
                        nc.sync.dma_start(cfs[:], cfd)

                with (
                    tc.tile_pool(name="vps", bufs=4, space="PSUM") as vps,
                    tc.tile_pool(name="qkps", bufs=2, space="PSUM") as qkps,
                ):

                    def emit_qk(h):
                        # two tq chunks share one 2-bank PSUM tile: the
                        # stationary weight chunk is reused across both
                        # matmuls and the rope multiply runs 1024 wide.
                        for wi, dst in ((0, qT), (1, kT)):
                            for tqp in range(NTQ // 2):
                                pm = qkps.tile([128, 2 * TQC], F32, tag="pm")
                                for cc in range(CCH):
                                    g, j = divmod(cc, CCG)
                                    woff = j * CW + T + wi * DG + h * D
                                    for half in range(2):
                                        tq = 2 * tqp + half
                                        nc.tensor.matmul(
                                            pm[:, half * TQC : (half + 1) * TQC],
                                            xwg[g][:, woff : woff + D],
                                            xwg[g][
                                                :,
                                                j * CW + tq * TQC : j * CW
                                                + (tq + 1) * TQC,
                                            ],
                                            start=(cc == 0),
                                            stop=(cc == CCH - 1),
                                        )
                                nc.vector.tensor_mul(
                                    dst[
                                        :,
                                        h * T + 2 * tqp * TQC : h * T
                                        + 2 * (tqp + 1) * TQC,
                                    ],
                                    pm[:],
                                    cfs[:, 2 * tqp * TQC : 2 * (tqp + 1) * TQC],
                                )

                    def emit_v():
                        for tk in range(NTK):
                            pv = vps.tile([128, DG], F32, tag="pv")
                            for cc in range(CCH):
                                g, j = divmod(cc, CCG)
                                nc.tensor.matmul(
                                    pv[:],
                                    xwg[g][
                                        :, j * CW + tk * 128 : j * CW + tk * 128 + 128
                                    ],
                                    xwg[g][:, j * CW + T + 2 * DG : (j + 1) * CW],
                                    start=(cc == 0),
                                    stop=(cc == CCH - 1),
                                )
                            nc.any.tensor_copy(vv[:, tk * DG : (tk + 1) * DG], pv[:])

                    emit_qk(0)
                    emit_v()
                    for h in range(1, HG):
                        emit_qk(h)

            # ---------------- Phase B: attention + output projection ----------
            with (
                tc.tile_pool(name="phb", bufs=1) as bp,
                tc.tile_pool(name="ep", bufs=6) as epool,
                tc.tile_pool(name="es", bufs=3) as espool,
                tc.tile_pool(name="rp", bufs=2) as rpool,
                tc.tile_pool(name="op", bufs=3) as opool,
                tc.tile_pool(name="mp", bufs=4) as mpool,
                tc.tile_pool(name="scps", bufs=2, space="PSUM") as scps,
                tc.tile_pool(name="o2ps", bufs=2, space="PSUM") as o2ps,
                tc.tile_pool(name="accps", bufs=2, space="PSUM") as accps,
            ):
                wos = bp.tile([128, HG * C], F16, tag="wos")
                nc.sync.dma_start(wos[:], wop)

                def attn_chunk(h, tq):
                    """Scores/exp/AV for one (head, tq) chunk. Two key tiles
                    share one 2-bank scores PSUM tile so the exp runs 1024
                    wide (amortizes ACT per-instruction overhead). Returns the
                    state needed by finalize(), which is emitted later (one
                    head behind) so the softmax tail chain (esum -> ones-mm ->
                    recip -> mul) never blocks the PE stream."""
                    o2p = o2ps.tile([128, TQC], F32, tag="o2p")
                    esum = espool.tile([128, TQC], F16, tag="esum")
                    qrhs = qT[:, h * T + tq * TQC : h * T + (tq + 1) * TQC]
                    pend = None
                    for tkp in range(NTK // 2):
                        scp = scps.tile([128, 2 * TQC], F32, tag="scp")
                        for half in range(2):
                            tk = 2 * tkp + half
                            nc.tensor.matmul(
                                scp[:, half * TQC : (half + 1) * TQC],
                                kT[:, h * T + tk * 128 : h * T + tk * 128 + 128],
                                qrhs,
                                start=True,
                                stop=True,
                            )
                        et = epool.tile([128, 2 * TQC], F16, tag="et")
                        if use_mask:
                            ma = mpool.tile([128, 2 * TQC], F32, tag="ma")
                            for half in range(2):
                                tk = 2 * tkp + half
                                mt = mpool.tile([128, TQC], F32, tag="mt")
                                nc.sync.dma_start(
                                    mt[:],
                                    maskT[
                                        tk * 128 : (tk + 1) * 128,
                                        tq * TQC : (tq + 1) * TQC,
                                    ],
                                )
                                nc.vector.tensor_add(
                                    ma[:, half * TQC : (half + 1) * TQC],
                                    scp[:, half * TQC : (half + 1) * TQC],
                                    mt[:],
                                )
                            nc.scalar.activation(et[:], ma[:], EXP)
                        else:
                            nc.scalar.activation(et[:], scp[:], EXP)
                        for half in range(2):
                            if tkp == 0 and half == 0:
                                nc.vector.tensor_copy(
                                    esum[:], et[:, half * TQC : (half + 1) * TQC]
                                )
                            else:
                                nc.vector.tensor_add(
                                    esum[:],
                                    esum[:],
                                    et[:, half * TQC : (half + 1) * TQC],
                                )
                        if pend is not None:
                            p_et, ptkp = pend
                            for half in range(2):
                                ptk = 2 * ptkp + half
                                nc.tensor.matmul(
                                    o2p[:],
                                    vv[:, ptk * DG + h * D : ptk * DG + (h + 1) * D],
                                    p_et[:, half * TQC : (half + 1) * TQC],
                                    start=(ptk == 0),
                                    stop=False,
                                )
                        pend = (et, tkp)
                    p_et, ptkp = pend
                    for half in range(2):
                        ptk = 2 * ptkp + half
                        nc.tensor.matmul(
                            o2p[:],
                            vv[:, ptk * DG + h * D : ptk * DG + (h + 1) * D],
                            p_et[:, half * TQC : (half + 1) * TQC],
                            start=False,
                            stop=(half == 1),
                        )
                    return (h, tq, o2p, esum)

                def finalize(state):
                    h, tq, o2p, esum = state
                    sp = accps.tile([128, TQC], F32, tag="acc")
                    nc.tensor.matmul(sp[:], ones[:], esum[:], start=True, stop=True)
                    rt = rpool.tile([128, TQC], F32, tag="rt")
                    nc.vector.reciprocal(rt[:], sp[:])
                    nc.vector.tensor_mul(
                        o2[:, h * T + tq * TQC : h * T + (tq + 1) * TQC],
                        o2p[:],
                        rt[:],
                    )

                def outproj_qt(qt):
                    ost = opool.tile([128, C], F16, tag="ost")
                    for oc in range(C // TQC):
                        fp = accps.tile([128, TQC], F32, tag="acc")
                        for h in range(HG):
                            nc.tensor.matmul(
                                fp[:],
                                o2[:, h * T + qt * 128 : h * T + qt * 128 + 128],
                                wos[:, h * C + oc * TQC : h * C + (oc + 1) * TQC],
                                start=(h == 0),
                                stop=(h == HG - 1),
                            )
                        nc.vector.tensor_copy(ost[:, oc * TQC : (oc + 1) * TQC], fp[:])
                    nc.gpsimd.dma_start(out[qt * 128 : (qt + 1) * 128, :], ost[:])

                # out-proj of tq chunk N is spread one row-tile per attention
                # chunk of tq N+1, so its matmuls fill PE slack while ACT
                # works on fresh exps; the last chunk's row-tiles drain at the
                # end.
                pending_fin = None
                for tq in range(NTQ):
                    for h in range(HG):
                        st = attn_chunk(h, tq)
                        if pending_fin is not None:
                            finalize(pending_fin)
                        pending_fin = st
                        if tq > 0:
                            outproj_qt((tq - 1) * 4 + h)
                    finalize(pending_fin)
                    pending_fin = None
                for qt in range((NTQ - 1) * 4, NTQ * 4):
                    outproj_qt(qt)

    nc.compile()
    return nc


def compute_cf(T, D, theta=THETA):
    """cf = (cos+sin).T * T**-0.25  [D, T] fp32.

    Both q and k are multiplied by cf, so scaling cf by T**-0.25 applies the
    module's 1/sqrt(T) logit scale symmetrically. Folding it here (instead of
    into Wq) keeps the fp16 weight values out of subnormal range."""
    freq = 1.0 / theta ** (np.arange(0, D, 2, dtype=np.float64) / D)
    t = np.arange(T, dtype=np.float64)
    m = np.einsum("i,j->ij", t, freq)  # [T, D/2]
    m = np.concatenate([m, m], axis=-1)  # [T, D]
    cfac = ((np.cos(m) + np.sin(m)) * T**-0.25).astype(np.float32)  # [T, D]
    return np.ascontiguousarray(cfac.T)  # [D, T]


_NC_CACHE = {}


def _get_nc(use_mask):
    key = bool(use_mask)
    if key not in _NC_CACHE:
        _NC_CACHE[key] = build_attention_nc(use_mask=key)
    return _NC_CACHE[key]


def _pack_xw(x_b, wq_g, wk_g, wv_g):
    """x [T, C] + three weights [C, DG] fp32 -> [128, CCH*(T+3*DG)] fp16,
    interleaved per contraction chunk: [x_cc (T) | wq_cc | wk_cc | wv_cc]."""
    xT = x_b.T.astype(np.float16).reshape(CCH, 128, SEQ)
    parts = [w.reshape(CCH, 128, DG).astype(np.float16) for w in (wq_g, wk_g, wv_g)]
    packed = np.concatenate([xT] + parts, axis=2)  # [CCH, 128, T+3*DG]
    cw = SEQ + 3 * DG
    return np.ascontiguousarray(packed.transpose(1, 0, 2).reshape(128, CCH * cw))


def _pack_wo(wo_g):
    """[DG, C] fp32 -> [128, HG*C] fp16 with [p, h*C + c] = wo[h*128+p, c]."""
    return np.ascontiguousarray(
        wo_g.reshape(HG, 128, HIDDEN).transpose(1, 0, 2).reshape(128, HG * HIDDEN)
    ).astype(np.float16)


def kernel(input_ids, attention_mask, Wq, Wk, Wv, Wo):
    input_ids = np.asarray(input_ids, dtype=np.float32)
    attention_mask = np.asarray(attention_mask, dtype=np.float32)
    Wq = np.asarray(Wq, dtype=np.float32)
    Wk = np.asarray(Wk, dtype=np.float32)
    Wv = np.asarray(Wv, dtype=np.float32)
    Wo = np.asarray(Wo, dtype=np.float32)

    b, t, c = input_ids.shape
    assert (b, t, c) == (BATCH, SEQ, HIDDEN)

    use_mask = bool(np.any(attention_mask))
    nc = _get_nc(use_mask)

    cf = compute_cf(SEQ, HEAD_DIM)

    masks = (
        [np.ascontiguousarray(attention_mask[bi, 0].T) for bi in range(BATCH)]
        if use_mask
        else None
    )
    xws = {}
    wos = []
    for g in range(MP):
        sl = slice(g * DG, (g + 1) * DG)
        for bi in range(BATCH):
            xws[bi, g] = _pack_xw(input_ids[bi], Wq[:, sl], Wk[:, sl], Wv[:, sl])
        wos.append(_pack_wo(Wo[sl, :]))

    in_maps = []
    for core in range(N_CORES):
        bi, g = divmod(core, MP)
        m = {
            "xw": xws[bi, g],
            "wop": wos[g],
            "cf": cf,
        }
        if use_mask:
            m["maskT"] = masks[bi]
        in_maps.append(m)

    res = bass_utils.run_bass_kernel_spmd(nc, in_maps, core_ids=list(range(N_CORES)))

    out = np.zeros((BATCH, SEQ, HIDDEN), dtype=np.float32)
    for bi in range(BATCH):
        acc = res.results[bi * MP]["out"].astype(np.float32)
        for g in range(1, MP):
            acc = acc + res.results[bi * MP + g]["out"].astype(np.float32)
        out[bi] = acc
    return out


# revision 12
# speedup vs baseline: 1.0153x; 1.0113x over previous
"""Trainium2 Bass kernel for LGeM self-attention (b=2, t=2048, c=2048, h=16, d=128).

Sharding: 8 cores = 2 (batch, data-parallel) x 4 (head-groups of 4 heads,
tensor-parallel 'mp'). Each core computes q/k/v projections for its 4 heads,
attention, and a partial output projection (its 512 rows of Wo); the host
sums the 4 mp-partials per batch in fp32.

v2 design (vs the fp32 DRAM-scratch baseline):
  - fp16 storage for every matmul operand (x, packed qkv weights, wo, q, k,
    v, exp(S), out2, out partials); fp32 PSUM accumulation throughout. Halves
    HBM traffic and SBUF footprint, enables fast weight load on the PE.
  - Everything stays in SBUF between phases - no DRAM scratch round trips.
  - Host packs x / wqkv / wo into the exact SBUF layouts, so the input side
    is ~34 large contiguous DMAs arriving at contraction-chunk granularity
    (accumulation chains advance as the stream lands); output is 16 row-block
    stores issued on the gpsimd (SWDGE) queue so they don't contend with the
    input HWDGE ring.
  - softmax denominator: running DVE fp16 accumulation of the exp tiles
    (esum += et), then a single ones[128,128] matmul per (head, tq-chunk)
    replicates the partition-sum across partitions (free broadcast for the
    reciprocal multiply). Saves 240 PE matmuls vs accumulating ones@et per
    key tile.
  - scores are built transposed, S_T[tk, tq] = kT_tile.T @ qT, so attn@v
    needs no transposes and the normalized context lands as out2T[d, tq] -
    exactly the lhsT the output projection wants.
  - rope here is q*(cos+sin) elementwise (the module's rotate_half is
    identity); the 1/sqrt(t) logit scale is folded into the shared cf tensor
    as T**-0.25 (applied to both q and k). softmax is computed without
    max-subtraction (logits are ~N(0, 0.25^2): exp never overflows).
  - loop order tq-outer / head-inner in attention, with the output
    projection for each tq chunk emitted right after its last head, so
    out-proj matmuls fill PE gaps while ACT works on the next chunk's exps.
"""

import sys

sys.path.insert(0, "/opt/trn_rl_repo")

import numpy as np

import concourse.bass as bass
import concourse.mybir as mybir
import concourse.tile as tile
from concourse import bacc, bass_utils

F32 = mybir.dt.float32
F16 = mybir.dt.float16

HIDDEN = 2048
HEADS = 16
HEAD_DIM = 128
SEQ = 2048
BATCH = 2
N_CORES = 8
MP = 4  # tensor-parallel cores per batch
HG = HEADS // MP  # heads per core
THETA = 10000.0

CCH = HIDDEN // 128  # 16 contraction chunks for projections
XG = 4  # x / w DMA groups
CCG = CCH // XG  # cc chunks per group
TQC = 512  # moving-dim chunk (tq)
NTQ = SEQ // TQC
NTK = SEQ // 128
DG = HG * HEAD_DIM  # 512 projection cols per core


def build_attention_nc(use_mask=False):
    T, C, D = SEQ, HIDDEN, HEAD_DIM
    EXP = mybir.ActivationFunctionType.Exp

    nc = bacc.Bacc("TRN2", target_bir_lowering=False, debug=False)

    # [p, cc*3584 + t] = x[t, cc*128+p] for t<2048, then the three weight
    # chunks W_wi[cc*128+p, dcol] (wi: 0=q,1=k,2=v) in the last 1536 cols
    CW = T + 3 * DG  # 3584 packed columns per cc chunk
    xw_d = nc.dram_tensor("xw", [128, CCH * CW], F16, kind="ExternalInput").ap()
    # [p, h*C + c] = Wo[h*128+p, c]
    wop = nc.dram_tensor("wop", [128, HG * C], F16, kind="ExternalInput").ap()
    cfd = nc.dram_tensor("cf", [D, T], F16, kind="ExternalInput").ap()
    if use_mask:
        maskT = nc.dram_tensor("maskT", [T, T], F32, kind="ExternalInput").ap()
    out = nc.dram_tensor("out", [T, C], F16, kind="ExternalOutput").ap()

    with tile.TileContext(nc) as tc:
        with tc.tile_pool(name="persist", bufs=1) as pp:
            qT = pp.tile([128, HG * T], F16, tag="qT")  # [d, h*T + t]
            kT = pp.tile([128, HG * T], F16, tag="kT")
            vv = pp.tile([128, NTK * DG], F16, tag="vv")  # [t%128, tk*DG + dg]
            o2 = pp.tile([128, HG * T], F16, tag="o2")  # [d, h*T + t]
            cfs = pp.tile([128, T], F16, tag="cfs")
            ones = pp.tile([128, 128], F16, tag="ones")
            nc.vector.memset(ones[:], 1.0)

            # ---------------- Phase A: projections ----------------
            with tc.tile_pool(name="xw", bufs=1) as xw:
                xwg = [
                    xw.tile([128, CCG * CW], F16, tag=f"xwg{g}", name=f"xwg{g}")
                    for g in range(XG)
                ]
                # cc-granular arrival so accumulation chains advance smoothly
                # while the input stream lands (avoids group-sized PE stalls)
                for cc in range(CCH):
                    g, j = divmod(cc, CCG)
                    nc.sync.dma_start(
                        xwg[g][:, j * CW : (j + 1) * CW],
                        xw_d[:, cc * CW : (cc + 1) * CW],
                    )
                    if cc == 1:
                        # cf is first needed by the rope multiply after the
                        # first 16-matmul chain; don't let it delay x0/w0
                        nc.sync.dma_start(cfs[:], cfd)

                with (
                    tc.tile_pool(name="vps", bufs=4, space="PSUM") as vps,
                    tc.tile_pool(name="qkps", bufs=2, space="PSUM") as qkps,
                ):

                    def emit_qk(h):
                        # two tq chunks share one 2-bank PSUM tile: the
                        # stationary weight chunk is reused across both
                        # matmuls and the rope multiply runs 1024 wide.
                        for wi, dst in ((0, qT), (1, kT)):
                            for tqp in range(NTQ // 2):
                                pm = qkps.tile([128, 2 * TQC], F32, tag="pm")
                                for cc in range(CCH):
                                    g, j = divmod(cc, CCG)
                                    woff = j * CW + T + wi * DG + h * D
                                    for half in range(2):
                                        tq = 2 * tqp + half
                                        nc.tensor.matmul(
                                            pm[:, half * TQC : (half + 1) * TQC],
                                            xwg[g][:, woff : woff + D],
                                            xwg[g][
                                                :,
                                                j * CW + tq * TQC : j * CW
                                                + (tq + 1) * TQC,
                                            ],
                                            start=(cc == 0),
                                            stop=(cc == CCH - 1),
                                        )
                                nc.vector.tensor_mul(
                                    dst[
                                        :,
                                        h * T + 2 * tqp * TQC : h * T
                                        + 2 * (tqp + 1) * TQC,
                                    ],
                                    pm[:],
                                    cfs[:, 2 * tqp * TQC : 2 * (tqp + 1) * TQC],
                                )

                    def emit_v():
                        for tk in range(NTK):
                            pv = vps.tile([128, DG], F32, tag="pv")
                            for cc in range(CCH):
                                g, j = divmod(cc, CCG)
                                nc.tensor.matmul(
                                    pv[:],
                                    xwg[g][
                                        :, j * CW + tk * 128 : j * CW + tk * 128 + 128
                                    ],
                                    xwg[g][:, j * CW + T + 2 * DG : (j + 1) * CW],
                                    start=(cc == 0),
                                    stop=(cc == CCH - 1),
                                )
                            nc.any.tensor_copy(vv[:, tk * DG : (tk + 1) * DG], pv[:])

                    emit_qk(0)
                    emit_v()
                    for h in range(1, HG):
                        emit_qk(h)

            # ---------------- Phase B: attention + output projection ----------
            with (
                tc.tile_pool(name="phb", bufs=1) as bp,
                tc.tile_pool(name="ep", bufs=6) as epool,
                tc.tile_pool(name="es", bufs=3) as espool,
                tc.tile_pool(name="rp", bufs=2) as rpool,
                tc.tile_pool(name="op", bufs=3) as opool,
                tc.tile_pool(name="mp", bufs=4) as mpool,
                tc.tile_pool(name="scps", bufs=2, space="PSUM") as scps,
                tc.tile_pool(name="o2ps", bufs=2, space="PSUM") as o2ps,
                tc.tile_pool(name="accps", bufs=2, space="PSUM") as accps,
            ):
                wos = bp.tile([128, HG * C], F16, tag="wos")
                nc.sync.dma_start(wos[:], wop)

                def attn_chunk(h, tq):
                    """Scores/exp/AV for one (head, tq) chunk. Two key tiles
                    share one 2-bank scores PSUM tile so the exp runs 1024
                    wide (amortizes ACT per-instruction overhead). Returns the
                    state needed by finalize(), which is emitted later (one
                    head behind) so the softmax tail chain (esum -> ones-mm ->
                    recip -> mul) never blocks the PE stream."""
                    o2p = o2ps.tile([128, TQC], F32, tag="o2p")
                    esum = espool.tile([128, TQC], F16, tag="esum")
                    qrhs = qT[:, h * T + tq * TQC : h * T + (tq + 1) * TQC]
                    pend = None
                    for tkp in range(NTK // 2):
                        scp = scps.tile([128, 2 * TQC], F32, tag="scp")
                        for half in range(2):
                            tk = 2 * tkp + half
                            nc.tensor.matmul(
                                scp[:, half * TQC : (half + 1) * TQC],
                                kT[:, h * T + tk * 128 : h * T + tk * 128 + 128],
                                qrhs,
                                start=True,
                                stop=True,
                            )
                        et = epool.tile([128, 2 * TQC], F16, tag="et")
                        if use_mask:
                            ma = mpool.tile([128, 2 * TQC], F32, tag="ma")
                            for half in range(2):
                                tk = 2 * tkp + half
                                mt = mpool.tile([128, TQC], F32, tag="mt")
                                nc.sync.dma_start(
                                    mt[:],
                                    maskT[
                                        tk * 128 : (tk + 1) * 128,
                                        tq * TQC : (tq + 1) * TQC,
                                    ],
                                )
                                nc.vector.tensor_add(
                                    ma[:, half * TQC : (half + 1) * TQC],
                                    scp[:, half * TQC : (half + 1) * TQC],
                                    mt[:],
                                )
                            nc.scalar.activation(et[:], ma[:], EXP)
                        else:
                            nc.scalar.activation(et[:], scp[:], EXP)
                        for half in range(2):
                            if tkp == 0 and half == 0:
                                nc.vector.tensor_copy(
                                    esum[:], et[:, half * TQC : (half + 1) * TQC]
                                )
                            else:
                                nc.vector.tensor_add(
                                    esum[:],
                                    esum[:],
                                    et[:, half * TQC : (half + 1) * TQC],
                                )
                        if pend is not None:
                            p_et, ptkp = pend
                            for half in range(2):
                                ptk = 2 * ptkp + half
                                nc.tensor.matmul(
                                    o2p[:],
                                    vv[:, ptk * DG + h * D : ptk * DG + (h + 1) * D],
                                    p_et[:, half * TQC : (half + 1) * TQC],
                                    start=(ptk == 0),
                                    stop=False,
                                )
                        pend = (et, tkp)
                    p_et, ptkp = pend
                    for half in range(2):
                        ptk = 2 * ptkp + half
                        nc.tensor.matmul(
                            o2p[:],
                            vv[:, ptk * DG + h * D : ptk * DG + (h + 1) * D],
                            p_et[:, half * TQC : (half + 1) * TQC],
                            start=False,
                            stop=(half == 1),
                        )
                    return (h, tq, o2p, esum)

                def finalize(state):
                    h, tq, o2p, esum = state
                    sp = accps.tile([128, TQC], F32, tag="acc")
                    nc.tensor.matmul(sp[:], ones[:], esum[:], start=True, stop=True)
                    rt = rpool.tile([128, TQC], F32, tag="rt")
                    nc.vector.reciprocal(rt[:], sp[:])
                    nc.vector.tensor_mul(
                        o2[:, h * T + tq * TQC : h * T + (tq + 1) * TQC],
                        o2p[:],
                        rt[:],
                    )

                def outproj_qt(qt):
                    ost = opool.tile([128, C], F16, tag="ost")
                    for oc in range(C // TQC):
                        fp = accps.tile([128, TQC], F32, tag="acc")
                        for h in range(HG):
                            nc.tensor.matmul(
                                fp[:],
                                o2[:, h * T + qt * 128 : h * T + qt * 128 + 128],
                                wos[:, h * C + oc * TQC : h * C + (oc + 1) * TQC],
                                start=(h == 0),
                                stop=(h == HG - 1),
                            )
                        nc.vector.tensor_copy(ost[:, oc * TQC : (oc + 1) * TQC], fp[:])
                    nc.gpsimd.dma_start(out[qt * 128 : (qt + 1) * 128, :], ost[:])

                # out-proj of tq chunk N is spread one row-tile per attention
                # chunk of tq N+1, so its matmuls fill PE slack while ACT
                # works on fresh exps; the last chunk's row-tiles drain at the
                # end.
                pending_fin = None
                for tq in range(NTQ):
                    for h in range(HG):
                        st = attn_chunk(h, tq)
                        if pending_fin is not None:
                            finalize(pending_fin)
                        pending_fin = st
                        if tq > 0:
                            outproj_qt((tq - 1) * 4 + h)
                    finalize(pending_fin)
                    pending_fin = None
                for qt in range((NTQ - 1) * 4, NTQ * 4):
                    outproj_qt(qt)

    nc.compile()
    return nc


def compute_cf(T, D, theta=THETA):
    """cf = (cos+sin).T * T**-0.25  [D, T] fp32.

    Both q and k are multiplied by cf, so scaling cf by T**-0.25 applies the
    module's 1/sqrt(T) logit scale symmetrically. Folding it here (instead of
    into Wq) keeps the fp16 weight values out of subnormal range."""
    freq = 1.0 / theta ** (np.arange(0, D, 2, dtype=np.float64) / D)
    t = np.arange(T, dtype=np.float64)
    m = np.einsum("i,j->ij", t, freq)  # [T, D/2]
    m = np.concatenate([m, m], axis=-1)  # [T, D]
    cfac = ((np.cos(m) + np.sin(m)) * T**-0.25).astype(np.float16)  # [T, D]
    return np.ascontiguousarray(cfac.T)  # [D, T]


_NC_CACHE = {}


def _get_nc(use_mask):
    key = bool(use_mask)
    if key not in _NC_CACHE:
        _NC_CACHE[key] = build_attention_nc(use_mask=key)
    return _NC_CACHE[key]


def _pack_xw(x_b, wq_g, wk_g, wv_g):
    """x [T, C] + three weights [C, DG] fp32 -> [128, CCH*(T+3*DG)] fp16,
    interleaved per contraction chunk: [x_cc (T) | wq_cc | wk_cc | wv_cc]."""
    xT = x_b.T.astype(np.float16).reshape(CCH, 128, SEQ)
    parts = [w.reshape(CCH, 128, DG).astype(np.float16) for w in (wq_g, wk_g, wv_g)]
    packed = np.concatenate([xT] + parts, axis=2)  # [CCH, 128, T+3*DG]
    cw = SEQ + 3 * DG
    return np.ascontiguousarray(packed.transpose(1, 0, 2).reshape(128, CCH * cw))


def _pack_wo(wo_g):
    """[DG, C] fp32 -> [128, HG*C] fp16 with [p, h*C + c] = wo[h*128+p, c]."""
    return np.ascontiguousarray(
        wo_g.reshape(HG, 128, HIDDEN).transpose(1, 0, 2).reshape(128, HG * HIDDEN)
    ).astype(np.float16)


def kernel(input_ids, attention_mask, Wq, Wk, Wv, Wo):
    input_ids = np.asarray(input_ids, dtype=np.float32)
    attention_mask = np.asarray(attention_mask, dtype=np.float32)
    Wq = np.asarray(Wq, dtype=np.float32)
    Wk = np.asarray(Wk, dtype=np.float32)
    Wv = np.asarray(Wv, dtype=np.float32)
    Wo = np.asarray(Wo, dtype=np.float32)

    b, t, c = input_ids.shape
    assert (b, t, c) == (BATCH, SEQ, HIDDEN)

    use_mask = bool(np.any(attention_mask))
    nc = _get_nc(use_mask)

    cf = compute_cf(SEQ, HEAD_DIM)

    masks = (
        [np.ascontiguousarray(attention_mask[bi, 0].T) for bi in range(BATCH)]
        if use_mask
        else None
    )
    xws = {}
    wos = []
    for g in range(MP):
        sl = slice(g * DG, (g + 1) * DG)
        for bi in range(BATCH):
            xws[bi, g] = _pack_xw(input_ids[bi], Wq[:, sl], Wk[:, sl], Wv[:, sl])
        wos.append(_pack_wo(Wo[sl, :]))

    in_maps = []
    for core in range(N_CORES):
        bi, g = divmod(core, MP)
        m = {
            "xw": xws[bi, g],
            "wop": wos[g],
            "cf": cf,
        }
        if use_mask:
            m["maskT"] = masks[bi]
        in_maps.append(m)

    res = bass_utils.run_bass_kernel_spmd(nc, in_maps, core_ids=list(range(N_CORES)))

    out = np.zeros((BATCH, SEQ, HIDDEN), dtype=np.float32)
    for bi in range(BATCH):
        acc = res.results[bi * MP]["out"].astype(np.float32)
        for g in range(1, MP):
            acc = acc + res.results[bi * MP + g]["out"].astype(np.float32)
        out[bi] = acc
    return out


# revision 13
# speedup vs baseline: 1.0153x; 1.0000x over previous
"""Trainium2 Bass kernel for LGeM self-attention (b=2, t=2048, c=2048, h=16, d=128).

Sharding: 8 cores = 2 (batch, data-parallel) x 4 (head-groups of 4 heads,
tensor-parallel 'mp'). Each core computes q/k/v projections for its 4 heads,
attention, and a partial output projection (its 512 rows of Wo); the host
sums the 4 mp-partials per batch in fp32.

v2 design (vs the fp32 DRAM-scratch baseline):
  - fp16 storage for every matmul operand (x, packed qkv weights, wo, q, k,
    v, exp(S), out2, out partials); fp32 PSUM accumulation throughout. Halves
    HBM traffic and SBUF footprint, enables fast weight load on the PE.
  - Everything stays in SBUF between phases - no DRAM scratch round trips.
  - Host packs x / wqkv / wo into the exact SBUF layouts, so the input side
    is ~34 large contiguous DMAs arriving at contraction-chunk granularity
    (accumulation chains advance as the stream lands); output is 16 row-block
    stores issued on the gpsimd (SWDGE) queue so they don't contend with the
    input HWDGE ring.
  - softmax denominator: running DVE fp16 accumulation of the exp tiles
    (esum += et), then a single ones[128,128] matmul per (head, tq-chunk)
    replicates the partition-sum across partitions (free broadcast for the
    reciprocal multiply). Saves 240 PE matmuls vs accumulating ones@et per
    key tile.
  - scores are built transposed, S_T[tk, tq] = kT_tile.T @ qT, so attn@v
    needs no transposes and the normalized context lands as out2T[d, tq] -
    exactly the lhsT the output projection wants.
  - rope here is q*(cos+sin) elementwise (the module's rotate_half is
    identity); the 1/sqrt(t) logit scale is folded into the shared cf tensor
    as T**-0.25 (applied to both q and k). softmax is computed without
    max-subtraction (logits are ~N(0, 0.25^2): exp never overflows).
  - loop order tq-outer / head-inner in attention, with the output
    projection for each tq chunk emitted right after its last head, so
    out-proj matmuls fill PE gaps while ACT works on the next chunk's exps.
"""

import sys
from contextlib import ExitStack

sys.path.insert(0, "/opt/trn_rl_repo")

import numpy as np

import concourse.bass as bass
import concourse.mybir as mybir
import concourse.tile as tile
from concourse import bacc, bass_utils

F32 = mybir.dt.float32
F16 = mybir.dt.float16

HIDDEN = 2048
HEADS = 16
HEAD_DIM = 128
SEQ = 2048
BATCH = 2
N_CORES = 8
MP = 4  # tensor-parallel cores per batch
HG = HEADS // MP  # heads per core
THETA = 10000.0

CCH = HIDDEN // 128  # 16 contraction chunks for projections
XG = 4  # x / w DMA groups
CCG = CCH // XG  # cc chunks per group
TQC = 512  # moving-dim chunk (tq)
NTQ = SEQ // TQC
NTK = SEQ // 128
DG = HG * HEAD_DIM  # 512 projection cols per core


def build_attention_nc(use_mask=False):
    T, C, D = SEQ, HIDDEN, HEAD_DIM
    EXP = mybir.ActivationFunctionType.Exp

    nc = bacc.Bacc("TRN2", target_bir_lowering=False, debug=False)

    # [p, cc*3584 + t] = x[t, cc*128+p] for t<2048, then the three weight
    # chunks W_wi[cc*128+p, dcol] (wi: 0=q,1=k,2=v) in the last 1536 cols
    CW = T + 3 * DG  # 3584 packed columns per cc chunk
    xw_d = nc.dram_tensor("xw", [128, CCH * CW], F16, kind="ExternalInput").ap()
    # [p, h*C + c] = Wo[h*128+p, c]
    wop = nc.dram_tensor("wop", [128, HG * C], F16, kind="ExternalInput").ap()
    cfd = nc.dram_tensor("cf", [D, T], F16, kind="ExternalInput").ap()
    if use_mask:
        maskT = nc.dram_tensor("maskT", [T, T], F32, kind="ExternalInput").ap()
    out = nc.dram_tensor("out", [T, C], F16, kind="ExternalOutput").ap()

    with tile.TileContext(nc) as tc:
        with tc.tile_pool(name="persist", bufs=1) as pp:
            qT = pp.tile([128, HG * T], F16, tag="qT")  # [d, h*T + t]
            kT = pp.tile([128, HG * T], F16, tag="kT")
            vv = pp.tile([128, NTK * DG], F16, tag="vv")  # [t%128, tk*DG + dg]
            o2 = pp.tile([128, HG * T], F16, tag="o2")  # [d, h*T + t]
            cfs = pp.tile([128, T], F16, tag="cfs")
            ones = pp.tile([128, 128], F16, tag="ones")
            nc.vector.memset(ones[:], 1.0)

            # ---------------- Phase A: projections ----------------
            # attention-phase SBUF pools open BEFORE xw so they outlive it
            # (xw is closed mid-attention, right after the deferred head-3
            # q/k projection finishes reading it)
            b1 = ExitStack()
            epool = b1.enter_context(tc.tile_pool(name="ep", bufs=5))
            espool = b1.enter_context(tc.tile_pool(name="es", bufs=3))
            rpool = b1.enter_context(tc.tile_pool(name="rp", bufs=2))
            mpool = None  # mask build: opened in b2 (after xw closes)
            xws = ExitStack()
            xw = xws.enter_context(tc.tile_pool(name="xw", bufs=1))
            if True:
                xwg = [
                    xw.tile([128, CCG * CW], F16, tag=f"xwg{g}", name=f"xwg{g}")
                    for g in range(XG)
                ]
                # cc-granular arrival so accumulation chains advance smoothly
                # while the input stream lands (avoids group-sized PE stalls)
                for cc in range(CCH):
                    g, j = divmod(cc, CCG)
                    nc.sync.dma_start(
                        xwg[g][:, j * CW : (j + 1) * CW],
                        xw_d[:, cc * CW : (cc + 1) * CW],
                    )
                    if cc == 1:
                        # cf is first needed by the rope multiply after the
                        # first 16-matmul chain; don't let it delay x0/w0
                        nc.sync.dma_start(cfs[:], cfd)

                with (
                    tc.tile_pool(name="vps", bufs=4, space="PSUM") as vps,
                    tc.tile_pool(name="qkps", bufs=2, space="PSUM") as qkps,
                ):

                    def emit_qk(h):
                        # two tq chunks share one 2-bank PSUM tile: the
                        # stationary weight chunk is reused across both
                        # matmuls and the rope multiply runs 1024 wide.
                        for wi, dst in ((0, qT), (1, kT)):
                            for tqp in range(NTQ // 2):
                                pm = qkps.tile([128, 2 * TQC], F32, tag="pm")
                                for cc in range(CCH):
                                    g, j = divmod(cc, CCG)
                                    woff = j * CW + T + wi * DG + h * D
                                    for half in range(2):
                                        tq = 2 * tqp + half
                                        nc.tensor.matmul(
                                            pm[:, half * TQC : (half + 1) * TQC],
                                            xwg[g][:, woff : woff + D],
                                            xwg[g][
                                                :,
                                                j * CW + tq * TQC : j * CW
                                                + (tq + 1) * TQC,
                                            ],
                                            start=(cc == 0),
                                            stop=(cc == CCH - 1),
                                        )
                                nc.vector.tensor_mul(
                                    dst[
                                        :,
                                        h * T + 2 * tqp * TQC : h * T
                                        + 2 * (tqp + 1) * TQC,
                                    ],
                                    pm[:],
                                    cfs[:, 2 * tqp * TQC : 2 * (tqp + 1) * TQC],
                                )

                    def emit_v():
                        for tk in range(NTK):
                            pv = vps.tile([128, DG], F32, tag="pv")
                            for cc in range(CCH):
                                g, j = divmod(cc, CCG)
                                nc.tensor.matmul(
                                    pv[:],
                                    xwg[g][
                                        :, j * CW + tk * 128 : j * CW + tk * 128 + 128
                                    ],
                                    xwg[g][:, j * CW + T + 2 * DG : (j + 1) * CW],
                                    start=(cc == 0),
                                    stop=(cc == CCH - 1),
                                )
                            nc.any.tensor_copy(vv[:, tk * DG : (tk + 1) * DG], pv[:])

                    emit_qk(0)
                    emit_v()
                    emit_qk(1)
                    emit_qk(2)
                    if use_mask:
                        # fallback path keeps the simple structure (the mask
                        # pools need the SBUF that the deferred-projection
                        # trick would keep pinned)
                        emit_qk(3)
                    # else: head 3's q/k projection is deferred into phase B
                    # as PE filler for the first tq chunk's attention

            # ---------------- Phase B: attention + output projection ----------
            bps = ExitStack()
            scps = bps.enter_context(tc.tile_pool(name="scps", bufs=2, space="PSUM"))
            o2ps = bps.enter_context(tc.tile_pool(name="o2ps", bufs=2, space="PSUM"))
            accps = bps.enter_context(tc.tile_pool(name="accps", bufs=2, space="PSUM"))
            if True:

                def emit_qk3_chain(wi, tq):
                    # one deferred head-3 projection chain on a 1-bank acc
                    # tile; pumped between tq=0 attention chunks as PE filler
                    dst = qT if wi == 0 else kT
                    pm = accps.tile([128, TQC], F32, tag="acc", name="pm3")
                    for cc in range(CCH):
                        g, j = divmod(cc, CCG)
                        woff = j * CW + T + wi * DG + 3 * D
                        nc.tensor.matmul(
                            pm[:],
                            xwg[g][:, woff : woff + D],
                            xwg[g][:, j * CW + tq * TQC : j * CW + (tq + 1) * TQC],
                            start=(cc == 0),
                            stop=(cc == CCH - 1),
                        )
                    nc.vector.tensor_mul(
                        dst[:, 3 * T + tq * TQC : 3 * T + (tq + 1) * TQC],
                        pm[:],
                        cfs[:, tq * TQC : (tq + 1) * TQC],
                    )

                def attn_chunk(h, tq):
                    """Scores/exp/AV for one (head, tq) chunk. Two key tiles
                    share one 2-bank scores PSUM tile so the exp runs 1024
                    wide (amortizes ACT per-instruction overhead). Returns the
                    state needed by finalize(), which is emitted later (one
                    head behind) so the softmax tail chain (esum -> ones-mm ->
                    recip -> mul) never blocks the PE stream."""
                    o2p = o2ps.tile([128, TQC], F32, tag="o2p")
                    esum = espool.tile([128, TQC], F16, tag="esum")
                    qrhs = qT[:, h * T + tq * TQC : h * T + (tq + 1) * TQC]
                    pend = None
                    for tkp in range(NTK // 2):
                        scp = scps.tile([128, 2 * TQC], F32, tag="scp")
                        for half in range(2):
                            tk = 2 * tkp + half
                            nc.tensor.matmul(
                                scp[:, half * TQC : (half + 1) * TQC],
                                kT[:, h * T + tk * 128 : h * T + tk * 128 + 128],
                                qrhs,
                                start=True,
                                stop=True,
                            )
                        et = epool.tile([128, 2 * TQC], F16, tag="et")
                        if use_mask:
                            ma = mpool.tile([128, 2 * TQC], F32, tag="ma")
                            for half in range(2):
                                tk = 2 * tkp + half
                                mt = mpool.tile([128, TQC], F32, tag="mt")
                                nc.sync.dma_start(
                                    mt[:],
                                    maskT[
                                        tk * 128 : (tk + 1) * 128,
                                        tq * TQC : (tq + 1) * TQC,
                                    ],
                                )
                                nc.vector.tensor_add(
                                    ma[:, half * TQC : (half + 1) * TQC],
                                    scp[:, half * TQC : (half + 1) * TQC],
                                    mt[:],
                                )
                            nc.scalar.activation(et[:], ma[:], EXP)
                        else:
                            nc.scalar.activation(et[:], scp[:], EXP)
                        for half in range(2):
                            if tkp == 0 and half == 0:
                                nc.vector.tensor_copy(
                                    esum[:], et[:, half * TQC : (half + 1) * TQC]
                                )
                            else:
                                nc.vector.tensor_add(
                                    esum[:],
                                    esum[:],
                                    et[:, half * TQC : (half + 1) * TQC],
                                )
                        if pend is not None:
                            p_et, ptkp = pend
                            for half in range(2):
                                ptk = 2 * ptkp + half
                                nc.tensor.matmul(
                                    o2p[:],
                                    vv[:, ptk * DG + h * D : ptk * DG + (h + 1) * D],
                                    p_et[:, half * TQC : (half + 1) * TQC],
                                    start=(ptk == 0),
                                    stop=False,
                                )
                        pend = (et, tkp)
                    p_et, ptkp = pend
                    for half in range(2):
                        ptk = 2 * ptkp + half
                        nc.tensor.matmul(
                            o2p[:],
                            vv[:, ptk * DG + h * D : ptk * DG + (h + 1) * D],
                            p_et[:, half * TQC : (half + 1) * TQC],
                            start=False,
                            stop=(half == 1),
                        )
                    return (h, tq, o2p, esum)

                def finalize(state):
                    h, tq, o2p, esum = state
                    sp = accps.tile([128, TQC], F32, tag="acc")
                    nc.tensor.matmul(sp[:], ones[:], esum[:], start=True, stop=True)
                    rt = rpool.tile([128, TQC], F32, tag="rt")
                    nc.vector.reciprocal(rt[:], sp[:])
                    nc.vector.tensor_mul(
                        o2[:, h * T + tq * TQC : h * T + (tq + 1) * TQC],
                        o2p[:],
                        rt[:],
                    )

                def outproj_qt(qt):
                    ost = opool.tile([128, C], F16, tag="ost")
                    for oc in range(C // TQC):
                        fp = accps.tile([128, TQC], F32, tag="acc")
                        for h in range(HG):
                            nc.tensor.matmul(
                                fp[:],
                                o2[:, h * T + qt * 128 : h * T + qt * 128 + 128],
                                wos[:, h * C + oc * TQC : h * C + (oc + 1) * TQC],
                                start=(h == 0),
                                stop=(h == HG - 1),
                            )
                        nc.vector.tensor_copy(ost[:, oc * TQC : (oc + 1) * TQC], fp[:])
                    nc.gpsimd.dma_start(out[qt * 128 : (qt + 1) * 128, :], ost[:])

                pending_fin = None
                if not use_mask:
                    # tq=0: deferred head-3 q/k chains fill PE slack (there
                    # is no out-proj filler yet); q chains first so chunk
                    # (3,0) has its rhs, k chains next for its lhsT tiles.
                    qk3 = [(0, t) for t in range(NTQ)]
                    qk3 += [(1, t) for t in range(NTQ)]
                    for h in range(HG):
                        if h == 3:
                            # chunk (3,0) reads the deferred head-3 q/k
                            # tiles: every chain must precede it
                            while qk3:
                                emit_qk3_chain(*qk3.pop(0))
                        st = attn_chunk(h, 0)
                        if pending_fin is not None:
                            finalize(pending_fin)
                        pending_fin = st
                        if h < 3:
                            for _ in range(3 if h else 2):
                                if qk3:
                                    emit_qk3_chain(*qk3.pop(0))
                    finalize(pending_fin)
                    pending_fin = None

                # projections (and for the fast path, tq=0) are done with the
                # packed input: release its SBUF for the wo / out-staging /
                # mask pools
                xws.close()
                b2 = ExitStack()
                bp = b2.enter_context(tc.tile_pool(name="phb", bufs=1))
                opool = b2.enter_context(tc.tile_pool(name="op", bufs=3))
                if use_mask:
                    mpool = b2.enter_context(tc.tile_pool(name="mp", bufs=4))
                wos = bp.tile([128, HG * C], F16, tag="wos")
                nc.sync.dma_start(wos[:], wop)

                # remaining tq chunks: out-proj of the previous tq chunk is
                # the PE filler
                tq0 = 1 if not use_mask else 0
                for tq in range(tq0, NTQ):
                    for h in range(HG):
                        st = attn_chunk(h, tq)
                        if pending_fin is not None:
                            finalize(pending_fin)
                        pending_fin = st
                        if tq > 0:
                            outproj_qt((tq - 1) * 4 + h)
                    finalize(pending_fin)
                    pending_fin = None
                for qt in range((NTQ - 1) * 4, NTQ * 4):
                    outproj_qt(qt)
                b2.close()
                bps.close()
                b1.close()

    nc.compile()
    return nc


def compute_cf(T, D, theta=THETA):
    """cf = (cos+sin).T * T**-0.25  [D, T] fp32.

    Both q and k are multiplied by cf, so scaling cf by T**-0.25 applies the
    module's 1/sqrt(T) logit scale symmetrically. Folding it here (instead of
    into Wq) keeps the fp16 weight values out of subnormal range."""
    freq = 1.0 / theta ** (np.arange(0, D, 2, dtype=np.float64) / D)
    t = np.arange(T, dtype=np.float64)
    m = np.einsum("i,j->ij", t, freq)  # [T, D/2]
    m = np.concatenate([m, m], axis=-1)  # [T, D]
    cfac = ((np.cos(m) + np.sin(m)) * T**-0.25).astype(np.float16)  # [T, D]
    return np.ascontiguousarray(cfac.T)  # [D, T]


_NC_CACHE = {}


def _get_nc(use_mask):
    key = bool(use_mask)
    if key not in _NC_CACHE:
        _NC_CACHE[key] = build_attention_nc(use_mask=key)
    return _NC_CACHE[key]


def _pack_xw(x_b, wq_g, wk_g, wv_g):
    """x [T, C] + three weights [C, DG] fp32 -> [128, CCH*(T+3*DG)] fp16,
    interleaved per contraction chunk: [x_cc (T) | wq_cc | wk_cc | wv_cc]."""
    xT = x_b.T.astype(np.float16).reshape(CCH, 128, SEQ)
    parts = [w.reshape(CCH, 128, DG).astype(np.float16) for w in (wq_g, wk_g, wv_g)]
    packed = np.concatenate([xT] + parts, axis=2)  # [CCH, 128, T+3*DG]
    cw = SEQ + 3 * DG
    return np.ascontiguousarray(packed.transpose(1, 0, 2).reshape(128, CCH * cw))


def _pack_wo(wo_g):
    """[DG, C] fp32 -> [128, HG*C] fp16 with [p, h*C + c] = wo[h*128+p, c]."""
    return np.ascontiguousarray(
        wo_g.reshape(HG, 128, HIDDEN).transpose(1, 0, 2).reshape(128, HG * HIDDEN)
    ).astype(np.float16)


def kernel(input_ids, attention_mask, Wq, Wk, Wv, Wo):
    input_ids = np.asarray(input_ids, dtype=np.float32)
    attention_mask = np.asarray(attention_mask, dtype=np.float32)
    Wq = np.asarray(Wq, dtype=np.float32)
    Wk = np.asarray(Wk, dtype=np.float32)
    Wv = np.asarray(Wv, dtype=np.float32)
    Wo = np.asarray(Wo, dtype=np.float32)

    b, t, c = input_ids.shape
    assert (b, t, c) == (BATCH, SEQ, HIDDEN)

    use_mask = bool(np.any(attention_mask))
    nc = _get_nc(use_mask)

    cf = compute_cf(SEQ, HEAD_DIM)

    masks = (
        [np.ascontiguousarray(attention_mask[bi, 0].T) for bi in range(BATCH)]
        if use_mask
        else None
    )
    xws = {}
    wos = []
    for g in range(MP):
        sl = slice(g * DG, (g + 1) * DG)
        for bi in range(BATCH):
            xws[bi, g] = _pack_xw(input_ids[bi], Wq[:, sl], Wk[:, sl], Wv[:, sl])
        wos.append(_pack_wo(Wo[sl, :]))

    in_maps = []
    for core in range(N_CORES):
        bi, g = divmod(core, MP)
        m = {
            "xw": xws[bi, g],
            "wop": wos[g],
            "cf": cf,
        }
        if use_mask:
            m["maskT"] = masks[bi]
        in_maps.append(m)

    res = bass_utils.run_bass_kernel_spmd(nc, in_maps, core_ids=list(range(N_CORES)))

    out = np.zeros((BATCH, SEQ, HIDDEN), dtype=np.float32)
    for bi in range(BATCH):
        acc = res.results[bi * MP]["out"].astype(np.float32)
        for g in range(1, MP):
            acc = acc + res.results[bi * MP + g]["out"].astype(np.float32)
        out[bi] = acc
    return out
